# revision 21
# baseline (speedup 1.0000x reference)
"""Trainium2 Bass kernel for nn_DCTExtractor.

Reference computation:
  - stego [8, 3, 1024, 1024] f32; per 8x8 block 2D DCT-II (D @ X @ D^T).
  - bits[i] = abs(round_half_even(dct[b,c,nh,nw,bh,bw])) % 2 for 1572864
    index tuples.
  - out [8, num_bits]: out[b_idx[i], i] = bits[i]; other rows 0.

Sharding: data-parallel over batch b across the 8 NeuronCores; core b
processes image b and produces output row b.

Fast path (canonical meshgrid indices: every (b,c,nh,nw) block contributes
positions (1,2),(2,1),(2,2),(3,1)): both DCT stages are fused into ONE PE
pass per 4-strip group via accumulating matmuls (contraction over k, the
column-within-block):

    F[q=(plane,nhl), m=(sg,wc,nwl)] = sum_k Wk^T @ X[:, m*8+k]
      Wk[8*nhl'+j, plane*16+nhl] = delta(nhl,nhl') * D[1+i',j] * D[1+bws,k]
      plane = i'*2 + bws; live planes 1..4 map to p = plane-1.

  The stationary operand is the small constant Wk (96 cols, cheap FWL
  loads); the image streams as the moving operand with free dim 512 (full
  1 cyc/col rate).  This eliminates the per-chunk LDWEIGHTS of image data
  that dominated the 2-stage version, plus the intermediate PSUM->SBUF
  copy and transposes.

  Precision: the PE multiplies at e10m11, so fp32 operands would be
  truncated to ~11-bit mantissas (1e-3 coeff error -> thousands of bit
  flips).  Instead the host splits X and W into fp16 hi+lo pairs
  (11+11-bit mantissas; fp16 products are exact in the e10m23 datapath)
  and the kernel accumulates 3 passes: Wh*Xh + Wh*Xl + Wl*Xh (the dropped
  Wl*Xl term is ~2e-6).  Measured coeff error ~5e-6 -> 0 bit flips.  The
  two fp16 images total the same 12 MiB of DMA as the original f32.

  Parity is 2 DVE ops per group: r = (F + 3*2^22) - 3*2^22 (RNE for
  |F| < 2^22, no abs needed) written as int32, then bits = r & 1 as int8
  (two's complement AND gives |n| mod 2 for negatives too).

General fallback (arbitrary indices): device computes the full 64-plane
parity table per image; host gathers bits and applies the b mask.
"""

import sys

if "/opt/trn_rl_repo" not in sys.path:
    sys.path.insert(0, "/opt/trn_rl_repo")

import numpy as np

BS = 8
B, C, H, W = 8, 3, 1024, 1024
NBH, NBW = H // BS, W // BS
POS = np.array([[1, 2], [2, 1], [2, 2], [3, 1]], dtype=np.int32)
NPOS = 4
SEG = C * NBH * NBW * NPOS  # bits per batch element = 196608
NUM_BITS = B * SEG
NSTRIP = C * (H // 128)  # 24 strips of 128 image rows per image
NGROUP = 6               # 4 strips per group
MAGIC = float(np.float32(8388608.0))    # 2^23 (general path, inputs >= 0)
MAGIC3 = float(np.float32(12582912.0))  # 3*2^22: RNE for |x| < 2^22, any sign

_CACHE = {}


def _split_sync_waits(nc):
    """The staged walrus build accepts at most ONE sync wait per
    instruction, but Tile's wait-assignment freely attaches several.
    Hoist all but the last wait of each instruction onto same-engine
    NoOps inserted directly before it (engines execute their stream in
    order, so the semantics are identical)."""
    from concourse import mybir

    if getattr(nc, "_sync_waits_split", False):
        return
    nc._sync_waits_split = True
    counter = 0
    for bb in nc.m.functions[0].blocks:
        out = []
        changed = False
        for inst in bb.instructions:
            si = inst.sync_info
            waits = list(si.on_wait) if si is not None else []
            if len(waits) > 1:
                for w in waits[:-1]:
                    nop = mybir.InstNoOp(
                        name=f"I-splitw-{counter}", ins=[], outs=[])
                    counter += 1
                    nop.engine = inst.engine
                    nop.sync_info = mybir.SyncInfo(on_update=[], on_wait=[w])
                    out.append(nop)
                si.on_wait = waits[-1:]
                changed = True
            out.append(inst)
        if changed:
            bb.instructions = out
    return


def _dct_matrix_f32() -> np.ndarray:
    k = np.arange(BS)[:, None].astype(np.float64)
    m = np.arange(BS)[None, :].astype(np.float64)
    D = np.cos(np.pi * (2.0 * m + 1.0) * k / (2.0 * BS)) * np.sqrt(2.0 / BS)
    D[0, :] = np.sqrt(1.0 / BS)
    return D.astype(np.float32)


def _canonical_indices():
    b, c, nh, nw, p = np.meshgrid(
        np.arange(B), np.arange(C), np.arange(NBH), np.arange(NBW),
        np.arange(NPOS), indexing="ij")
    return {
        "b_idx": b.reshape(-1).astype(np.int32),
        "c_idx": c.reshape(-1).astype(np.int32),
        "nh_idx": nh.reshape(-1).astype(np.int32),
        "nw_idx": nw.reshape(-1).astype(np.int32),
        "bh_idx": POS[p.reshape(-1), 0].astype(np.int32),
        "bw_idx": POS[p.reshape(-1), 1].astype(np.int32),
    }


def _is_canonical(b_idx, c_idx, nh_idx, nw_idx, bh_idx, bw_idx) -> bool:
    if b_idx.shape[0] != NUM_BITS:
        return False
    canon = _CACHE.setdefault("canon", _canonical_indices())
    got = {"b_idx": b_idx, "c_idx": c_idx, "nh_idx": nh_idx,
           "nw_idx": nw_idx, "bh_idx": bh_idx, "bw_idx": bw_idx}
    return all(np.array_equal(np.asarray(got[k]), canon[k]) for k in canon)


def _build_consts_fast():
    """WC [128, 8, 2, 96] fp16: for each k, hi/lo split of
    Wk[8*nhl+j, plane*16+nhl] = D[1+i',j] * D[1+bws,k] with
    plane = i'*2 + bws (i' in 0..2, bws in 0..1).  Live planes are 1..4
    (p = plane-1); planes 0 and 5 are computed but unused."""
    D = _dct_matrix_f32()
    BRC = np.zeros((128, 8, 96), dtype=np.float32)
    for nhl in range(16):
        for ip in range(3):
            for bws in range(2):
                plane = ip * 2 + bws
                # outer product over (j, k)
                BRC[8 * nhl:8 * nhl + 8, :, plane * 16 + nhl] = (
                    D[1 + ip, :][:, None] * D[1 + bws, :][None, :])
    Wh = BRC.astype(np.float16)
    Wl = (BRC - Wh.astype(np.float32)).astype(np.float16)
    # pad free dim 96 -> 128 so FWL (fast weight load) triggers
    WC = np.zeros((128, 8, 2, 128), dtype=np.float16)
    WC[:, :, 0, :96] = Wh
    WC[:, :, 1, :96] = Wl
    return np.ascontiguousarray(WC.reshape(128, 8 * 2 * 128))


def _build_consts_general():
    D = _dct_matrix_f32()
    BR8 = np.zeros((128, 128), dtype=np.float32)
    for nhl in range(16):
        BR8[nhl * 8:(nhl + 1) * 8, nhl * 8:(nhl + 1) * 8] = D.T  # [j, i]
    BC8 = np.zeros((128, 128), dtype=np.float32)
    for l in range(8):
        for nwl in range(16):
            BC8[nwl * 8:(nwl + 1) * 8, l * 16 + nwl] = D[l, :]
    return BR8, BC8


def _parity_ops(nc, pk, hk):
    """pk holds |coeff|. Compute parity = |RNE(pk)| mod 2 into pk using only
    add/sub/mul/abs (general path):
      r  = (pk + 2^23) - 2^23        round-half-even to integer
      h  = r * 0.5
      rh = (h + 2^23) - 2^23         = h if r even, else nearest even int
      parity = 2*|h - rh|            0.0 or 1.0
    Each step is its own instruction so every intermediate is rounded f32."""
    from concourse import mybir

    ts = nc.vector.tensor_scalar
    add, sub, mult = (mybir.AluOpType.add, mybir.AluOpType.subtract,
                      mybir.AluOpType.mult)
    ts(out=pk[:], in0=pk[:], scalar1=MAGIC, scalar2=None, op0=add)
    ts(out=pk[:], in0=pk[:], scalar1=MAGIC, scalar2=None, op0=sub)
    ts(out=hk[:], in0=pk[:], scalar1=0.5, scalar2=None, op0=mult)
    ts(out=pk[:], in0=hk[:], scalar1=MAGIC, scalar2=None, op0=add)
    ts(out=pk[:], in0=pk[:], scalar1=MAGIC, scalar2=None, op0=sub)
    nc.vector.tensor_tensor(
        out=pk[:], in0=hk[:], in1=pk[:], op=sub)
    nc.scalar.activation(
        out=pk[:], in_=pk[:], func=mybir.ActivationFunctionType.Abs,
        scale=2.0)


def build_fast_nc():
    """Per-core program: xh/xl [3,1024,1024] fp16 -> o [6, 96, 512] int8.

    o[g, plane*16 + nhl, sg*128 + wc*16 + nwl] = parity of dct coeff
    (bh = 1+i', bw = 1+bws, plane = i'*2+bws) of block (c, nh, nw) where
    strip s = 4*g + sg = c*8 + hg, nh = hg*16 + nhl, nw = wc*16 + nwl.
    """
    import concourse.bass as bass
    import concourse.tile as tile
    from concourse import mybir

    f16 = mybir.dt.float16
    f32 = mybir.dt.float32
    i16 = mybir.dt.int16
    nc = bass.Bass()
    # xh/xl rows are HOST-PRESHUFFLED to (k, m) order: element [c, h, k*128+m]
    # holds original pixel [c, h, m*8+k], so each k-slice is contiguous.
    xh = nc.dram_tensor("xh", [C, H, W], f16, kind="ExternalInput")
    xl = nc.dram_tensor("xl", [C, H, W], f16, kind="ExternalInput")
    wc = nc.dram_tensor("wc", [128, 8 * 2 * 128], f16, kind="ExternalInput")
    o = nc.dram_tensor("o", [NSTRIP, 96, 128], i16, kind="ExternalOutput")

    # (channel, first strip-in-channel, n strips): the edges are split into
    # 1/2-strip groups — the first so compute starts after only 0.25 MB of
    # DMA, the last so the final parity/store tail is short.
    groups = [(0, 0, 1), (0, 1, 1), (0, 2, 2)]
    groups += [(c, hb, 4) for c in range(C) for hb in (0, 4)][1:-1]
    groups += [(2, 4, 2), (2, 6, 1), (2, 7, 1)]

    with tile.TileContext(nc) as tc:
        with (
            tc.tile_pool(name="consts", bufs=1) as consts,
            tc.tile_pool(name="xs", bufs=2 * C) as xpool,
            tc.tile_pool(name="rt", bufs=2) as rpool,
            tc.tile_pool(name="bt", bufs=2) as bpool,
            tc.tile_pool(name="fp", bufs=4, space="PSUM") as fppool,
        ):
            wct = consts.tile([128, 8 * 2 * 128], f16)
            nc.sync.dma_start(out=wct[:], in_=wc[:, :])

            # per-group DMAs in group order, xh before xl, so each group's
            # Xh matmuls can start before its Xl piece has even landed.
            xch, xcl = [], []
            for c in range(C):
                xthc = xpool.tile([128, 8192], f16, tag="xs", name=f"xh{c}")
                xch.append(xthc)
                xtlc = xpool.tile([128, 8192], f16, tag="xs", name=f"xl{c}")
                xcl.append(xtlc)
            for c, hb, n in groups:
                rows = slice(hb * 128, (hb + n) * 128)
                for src, tiles in ((xh, xch), (xl, xcl)):
                    nc.sync.dma_start(
                        out=tiles[c][:].rearrange(
                            "p (t w) -> p t w", t=8)[:, hb:hb + n, :],
                        in_=src[c, rows, :].rearrange("(t p) w -> p t w", p=128))

            def wslice(k, hl):
                return wct[:, (k * 2 + hl) * 128:(k * 2 + hl + 1) * 128]

            s0 = 0
            for c, hb, n in groups:
                fp = fppool.tile([128, 128 * n], f32, tag="fp")
                # channel tile viewed as [128, t=8, k=8, m=128]; the n
                # strips t=hb..hb+n-1 of this group at fixed k give 128*n
                # contiguous-run columns (sg, m).
                xrh = xch[c][:].rearrange("p (t k m) -> p t k m", t=8, k=8)
                xrl = xcl[c][:].rearrange("p (t k m) -> p t k m", t=8, k=8)
                # accumulate Wh*Xh + Wh*Xl + Wl*Xh over all k, Xh passes
                # first (Xl may still be in flight)
                for k in range(8):
                    nc.tensor.matmul(
                        out=fp[:, :], lhsT=wslice(k, 0),
                        rhs=xrh[:, hb:hb + n, k, :],
                        start=(k == 0), stop=False)
                for k in range(8):
                    nc.tensor.matmul(
                        out=fp[:, :], lhsT=wslice(k, 1),
                        rhs=xrh[:, hb:hb + n, k, :],
                        start=False, stop=False)
                for k in range(8):
                    nc.tensor.matmul(
                        out=fp[:, :], lhsT=wslice(k, 0),
                        rhs=xrl[:, hb:hb + n, k, :],
                        start=False, stop=(k == 7))
                # parity: RNE via 3*2^22 magic (sign-safe), then &1
                # (bitVec ops cannot cast, so both tiles are int16)
                rt = rpool.tile([128, 128 * n], i16, tag="rt")
                nc.vector.tensor_scalar(
                    out=rt[0:96, :], in0=fp[0:96, :],
                    scalar1=MAGIC3, scalar2=MAGIC3,
                    op0=mybir.AluOpType.add, op1=mybir.AluOpType.subtract)
                bt = bpool.tile([128, 128 * n], i16, tag="bt")
                nc.vector.tensor_scalar(
                    out=bt[0:96, :], in0=rt[0:96, :],
                    scalar1=1, scalar2=None,
                    op0=mybir.AluOpType.bitwise_and)
                # o[s, q, m] <- bt[q, sg*128 + m] for s = s0 + sg; the DRAM
                # side is permuted to partition-major order to match SBUF
                nc.sync.dma_start(
                    out=o[s0:s0 + n].rearrange("s q m -> q s m"),
                    in_=bt[0:96, :].rearrange("q (s m) -> q s m", s=n))
                s0 += n
    return nc


def build_general_nc(nstrip=NSTRIP):
    """Per-core program: full 64-plane parity table.

    table [nstrip, 128, 1024] f32 where
    table[s=(c,hg), l*16+nwl, wc*128 + nhl*8 + i] =
        parity of dct coeff (bh=i, bw=l) of block (c, hg*16+nhl, wc*16+nwl).
    """
    import concourse.bass as bass
    import concourse.tile as tile
    from concourse import mybir

    f32 = mybir.dt.float32
    nc = bass.Bass()
    x = nc.dram_tensor("x", [C, H, W], f32, kind="ExternalInput")
    br = nc.dram_tensor("br", [128, 128], f32, kind="ExternalInput")
    bc = nc.dram_tensor("bc", [128, 128], f32, kind="ExternalInput")
    o = nc.dram_tensor("o", [nstrip, 128, 1024], f32, kind="ExternalOutput")

    with tile.TileContext(nc) as tc:
        with (
            tc.tile_pool(name="consts", bufs=1) as consts,
            tc.tile_pool(name="xs", bufs=2) as xpool,
            tc.tile_pool(name="ysb", bufs=2) as ypool,
            tc.tile_pool(name="pk", bufs=2) as pkpool,
            tc.tile_pool(name="yp", bufs=4, space="PSUM") as yppool,
            tc.tile_pool(name="fp", bufs=4, space="PSUM") as fppool,
        ):
            brt = consts.tile([128, 128], f32)
            nc.sync.dma_start(out=brt[:], in_=br[:, :])
            bct = consts.tile([128, 128], f32)
            nc.sync.dma_start(out=bct[:], in_=bc[:, :])

            for s in range(nstrip):
                c, hg = divmod(s, H // 128)
                xs = xpool.tile([128, 1024], f32, tag="xs")
                nc.sync.dma_start(
                    out=xs[:], in_=x[c, hg * 128:(hg + 1) * 128, :])
                ysb = ypool.tile([128, 1024], f32, tag="ysb")
                for wc in range(8):
                    yp = yppool.tile([128, 128], f32, tag="yp")
                    nc.tensor.matmul(
                        out=yp[:],
                        lhsT=xs[:, wc * 128:(wc + 1) * 128],
                        rhs=brt[:],
                        start=True, stop=True)
                    nc.vector.tensor_copy(
                        out=ysb[:, wc * 128:(wc + 1) * 128], in_=yp[:])
                pk = pkpool.tile([128, 1024], f32, tag="pk")
                hk = pkpool.tile([128, 1024], f32, tag="hk")
                for wc in range(8):
                    fp = fppool.tile([128, 128], f32, tag="fp")
                    nc.tensor.matmul(
                        out=fp[:],
                        lhsT=bct[:],
                        rhs=ysb[:, wc * 128:(wc + 1) * 128],
                        start=True, stop=True)
                    nc.scalar.activation(
                        out=pk[:, wc * 128:(wc + 1) * 128], in_=fp[:],
                        func=mybir.ActivationFunctionType.Abs)
                _parity_ops(nc, pk, hk)
                nc.sync.dma_start(out=o[s], in_=pk[:])
    return nc


def _run_spmd(nc, in_maps, trace=False):
    from concourse.bass_utils import run_bass_kernel_spmd

    _split_sync_waits(nc)

    res = run_bass_kernel_spmd(
        nc, in_maps, core_ids=list(range(B)), trace=trace)
    _CACHE["last_results"] = res
    return res.results


def _fast_path(stego, trace=False):
    key = "fast_nc"
    if key not in _CACHE:
        _CACHE[key] = build_fast_nc()
    nc = _CACHE[key]
    WC = _CACHE.setdefault("consts_fast", _build_consts_fast())
    Xh = stego.astype(np.float16)
    Xl = (stego - Xh.astype(np.float32)).astype(np.float16)
    # shuffle each row from w = m*8+k order to (k, m) order so the
    # device-side k-slices are contiguous
    Xh = np.ascontiguousarray(
        Xh.reshape(B, C, H, W // 8, 8).swapaxes(-1, -2)).reshape(B, C, H, W)
    Xl = np.ascontiguousarray(
        Xl.reshape(B, C, H, W // 8, 8).swapaxes(-1, -2)).reshape(B, C, H, W)
    in_maps = [
        {"xh": Xh[b], "xl": Xl[b], "wc": WC}
        for b in range(B)
    ]
    results = _run_spmd(nc, in_maps, trace=trace)
    out = np.zeros((B, NUM_BITS), dtype=np.float32)
    for b in range(B):
        O = results[b]["o"]  # [24, 96, 128] int16
        live = O[:, 16:80, :]                       # planes 1..4 -> p 0..3
        a = live.reshape(NSTRIP, 4, 16, 8, 16)      # s, p, nhl, wc, nwl
        a = a.transpose(0, 2, 3, 4, 1)              # s, nhl, wc, nwl, p
        seg = a.reshape(-1).astype(np.float32)      # (c, nh, nw, p) flat
        out[b, b * SEG:(b + 1) * SEG] = seg
    return out


def _general_path(stego, b_idx, c_idx, nh_idx, nw_idx, bh_idx, bw_idx,
                  trace=False):
    key = "general_nc"
    if key not in _CACHE:
        _CACHE[key] = build_general_nc()
    nc = _CACHE[key]
    BR8, BC8 = _CACHE.setdefault("consts_general", _build_consts_general())
    in_maps = [
        {"x": np.ascontiguousarray(stego[b]), "br": BR8, "bc": BC8}
        for b in range(B)
    ]
    results = _run_spmd(nc, in_maps, trace=trace)

    b_idx = np.asarray(b_idx).astype(np.int64)
    c_idx = np.asarray(c_idx).astype(np.int64)
    nh_idx = np.asarray(nh_idx).astype(np.int64)
    nw_idx = np.asarray(nw_idx).astype(np.int64)
    bh_idx = np.asarray(bh_idx).astype(np.int64)
    bw_idx = np.asarray(bw_idx).astype(np.int64)
    num_bits = b_idx.shape[0]

    # table[s=(c,hg), l*16+nwl, wc*128 + nhl*8 + i]
    s = c_idx * 8 + nh_idx // 16
    part = bw_idx * 16 + nw_idx % 16
    free = (nw_idx // 16) * 128 + (nh_idx % 16) * 8 + bh_idx
    flat = (s * 128 + part) * 1024 + free

    out = np.zeros((B, num_bits), dtype=np.float32)
    cols = np.arange(num_bits)
    for b in range(B):
        tb = results[b]["o"].reshape(-1)
        mask = b_idx == b
        out[b, cols[mask]] = tb[flat[mask]]
    return out


def kernel(stego, b_idx, c_idx, nh_idx, nw_idx, bh_idx, bw_idx):
    stego = np.ascontiguousarray(np.asarray(stego, dtype=np.float32))
    import os
    trace = os.environ.get("BASS_TRACE", "") not in ("", "0")
    if _is_canonical(b_idx, c_idx, nh_idx, nw_idx, bh_idx, bw_idx):
        return _fast_path(stego, trace=trace)
    return _general_path(
        stego, b_idx, c_idx, nh_idx, nw_idx, bh_idx, bw_idx, trace=trace)


# revision 22
# speedup vs baseline: 1.0262x; 1.0262x over previous
"""Trainium2 Bass kernel for nn_DCTExtractor.

Reference computation:
  - stego [8, 3, 1024, 1024] f32; per 8x8 block 2D DCT-II (D @ X @ D^T).
  - bits[i] = abs(round_half_even(dct[b,c,nh,nw,bh,bw])) % 2 for 1572864
    index tuples.
  - out [8, num_bits]: out[b_idx[i], i] = bits[i]; other rows 0.

Sharding: data-parallel over batch b across the 8 NeuronCores; core b
processes image b and produces output row b.

Fast path (canonical meshgrid indices: every (b,c,nh,nw) block contributes
positions (1,2),(2,1),(2,2),(3,1)): both DCT stages are fused into ONE PE
pass per 4-strip group via accumulating matmuls (contraction over k, the
column-within-block):

    F[q=(plane,nhl), m=(sg,wc,nwl)] = sum_k Wk^T @ X[:, m*8+k]
      Wk[8*nhl'+j, plane*16+nhl] = delta(nhl,nhl') * D[1+i',j] * D[1+bws,k]
      plane = i'*2 + bws; live planes 1..4 map to p = plane-1.

  The stationary operand is the small constant Wk (96 cols, cheap FWL
  loads); the image streams as the moving operand with free dim 512 (full
  1 cyc/col rate).  This eliminates the per-chunk LDWEIGHTS of image data
  that dominated the 2-stage version, plus the intermediate PSUM->SBUF
  copy and transposes.

  Precision: the PE multiplies at e10m11, so fp32 operands would be
  truncated to ~11-bit mantissas (1e-3 coeff error -> thousands of bit
  flips).  Instead the host splits X and W into fp16 hi+lo pairs
  (11+11-bit mantissas; fp16 products are exact in the e10m23 datapath)
  and the kernel accumulates 3 passes: Wh*Xh + Wh*Xl + Wl*Xh (the dropped
  Wl*Xl term is ~2e-6).  Measured coeff error ~5e-6 -> 0 bit flips.  The
  two fp16 images total the same 12 MiB of DMA as the original f32.

  Parity is 2 DVE ops per group: r = (F + 3*2^22) - 3*2^22 (RNE for
  |F| < 2^22, no abs needed) written as int32, then bits = r & 1 as int8
  (two's complement AND gives |n| mod 2 for negatives too).

General fallback (arbitrary indices): device computes the full 64-plane
parity table per image; host gathers bits and applies the b mask.
"""

import sys

if "/opt/trn_rl_repo" not in sys.path:
    sys.path.insert(0, "/opt/trn_rl_repo")

import numpy as np

BS = 8
B, C, H, W = 8, 3, 1024, 1024
NBH, NBW = H // BS, W // BS
POS = np.array([[1, 2], [2, 1], [2, 2], [3, 1]], dtype=np.int32)
NPOS = 4
SEG = C * NBH * NBW * NPOS  # bits per batch element = 196608
NUM_BITS = B * SEG
NSTRIP = C * (H // 128)  # 24 strips of 128 image rows per image
NGROUP = 6               # 4 strips per group
MAGIC = float(np.float32(8388608.0))    # 2^23 (general path, inputs >= 0)
MAGIC3 = float(np.float32(12582912.0))  # 3*2^22: RNE for |x| < 2^22, any sign

_CACHE = {}


def _split_sync_waits(nc):
    """The staged walrus build accepts at most ONE sync wait per
    instruction, but Tile's wait-assignment freely attaches several.
    Hoist all but the last wait of each instruction onto same-engine
    NoOps inserted directly before it (engines execute their stream in
    order, so the semantics are identical)."""
    from concourse import mybir

    if getattr(nc, "_sync_waits_split", False):
        return
    nc._sync_waits_split = True
    counter = 0
    for bb in nc.m.functions[0].blocks:
        out = []
        changed = False
        for inst in bb.instructions:
            si = inst.sync_info
            waits = list(si.on_wait) if si is not None else []
            if len(waits) > 1:
                for w in waits[:-1]:
                    nop = mybir.InstNoOp(
                        name=f"I-splitw-{counter}", ins=[], outs=[])
                    counter += 1
                    nop.engine = inst.engine
                    nop.sync_info = mybir.SyncInfo(on_update=[], on_wait=[w])
                    out.append(nop)
                si.on_wait = waits[-1:]
                changed = True
            out.append(inst)
        if changed:
            bb.instructions = out
    return


def _dct_matrix_f32() -> np.ndarray:
    k = np.arange(BS)[:, None].astype(np.float64)
    m = np.arange(BS)[None, :].astype(np.float64)
    D = np.cos(np.pi * (2.0 * m + 1.0) * k / (2.0 * BS)) * np.sqrt(2.0 / BS)
    D[0, :] = np.sqrt(1.0 / BS)
    return D.astype(np.float32)


def _canonical_indices():
    b, c, nh, nw, p = np.meshgrid(
        np.arange(B), np.arange(C), np.arange(NBH), np.arange(NBW),
        np.arange(NPOS), indexing="ij")
    return {
        "b_idx": b.reshape(-1).astype(np.int32),
        "c_idx": c.reshape(-1).astype(np.int32),
        "nh_idx": nh.reshape(-1).astype(np.int32),
        "nw_idx": nw.reshape(-1).astype(np.int32),
        "bh_idx": POS[p.reshape(-1), 0].astype(np.int32),
        "bw_idx": POS[p.reshape(-1), 1].astype(np.int32),
    }


def _is_canonical(b_idx, c_idx, nh_idx, nw_idx, bh_idx, bw_idx) -> bool:
    if b_idx.shape[0] != NUM_BITS:
        return False
    canon = _CACHE.setdefault("canon", _canonical_indices())
    got = {"b_idx": b_idx, "c_idx": c_idx, "nh_idx": nh_idx,
           "nw_idx": nw_idx, "bh_idx": bh_idx, "bw_idx": bw_idx}
    return all(np.array_equal(np.asarray(got[k]), canon[k]) for k in canon)


def _build_consts_fast():
    """WC [128, 8, 2, 96] fp16: for each k, hi/lo split of
    Wk[8*nhl+j, plane*16+nhl] = D[1+i',j] * D[1+bws,k] with
    plane = i'*2 + bws (i' in 0..2, bws in 0..1).  Live planes are 1..4
    (p = plane-1); planes 0 and 5 are computed but unused."""
    D = _dct_matrix_f32()
    BRC = np.zeros((128, 8, 96), dtype=np.float32)
    for nhl in range(16):
        for ip in range(3):
            for bws in range(2):
                plane = ip * 2 + bws
                # outer product over (j, k)
                BRC[8 * nhl:8 * nhl + 8, :, plane * 16 + nhl] = (
                    D[1 + ip, :][:, None] * D[1 + bws, :][None, :])
    Wh = BRC.astype(np.float16)
    Wl = (BRC - Wh.astype(np.float32)).astype(np.float16)
    # pad free dim 96 -> 128 so FWL (fast weight load) triggers
    WC = np.zeros((128, 8, 2, 128), dtype=np.float16)
    WC[:, :, 0, :96] = Wh
    WC[:, :, 1, :96] = Wl
    return np.ascontiguousarray(WC.reshape(128, 8 * 2 * 128))


def _build_consts_general():
    D = _dct_matrix_f32()
    BR8 = np.zeros((128, 128), dtype=np.float32)
    for nhl in range(16):
        BR8[nhl * 8:(nhl + 1) * 8, nhl * 8:(nhl + 1) * 8] = D.T  # [j, i]
    BC8 = np.zeros((128, 128), dtype=np.float32)
    for l in range(8):
        for nwl in range(16):
            BC8[nwl * 8:(nwl + 1) * 8, l * 16 + nwl] = D[l, :]
    return BR8, BC8


def _parity_ops(nc, pk, hk):
    """pk holds |coeff|. Compute parity = |RNE(pk)| mod 2 into pk using only
    add/sub/mul/abs (general path):
      r  = (pk + 2^23) - 2^23        round-half-even to integer
      h  = r * 0.5
      rh = (h + 2^23) - 2^23         = h if r even, else nearest even int
      parity = 2*|h - rh|            0.0 or 1.0
    Each step is its own instruction so every intermediate is rounded f32."""
    from concourse import mybir

    ts = nc.vector.tensor_scalar
    add, sub, mult = (mybir.AluOpType.add, mybir.AluOpType.subtract,
                      mybir.AluOpType.mult)
    ts(out=pk[:], in0=pk[:], scalar1=MAGIC, scalar2=None, op0=add)
    ts(out=pk[:], in0=pk[:], scalar1=MAGIC, scalar2=None, op0=sub)
    ts(out=hk[:], in0=pk[:], scalar1=0.5, scalar2=None, op0=mult)
    ts(out=pk[:], in0=hk[:], scalar1=MAGIC, scalar2=None, op0=add)
    ts(out=pk[:], in0=pk[:], scalar1=MAGIC, scalar2=None, op0=sub)
    nc.vector.tensor_tensor(
        out=pk[:], in0=hk[:], in1=pk[:], op=sub)
    nc.scalar.activation(
        out=pk[:], in_=pk[:], func=mybir.ActivationFunctionType.Abs,
        scale=2.0)


def build_fast_nc():
    """Per-core program: xh/xl [3,1024,1024] fp16 -> o [6, 96, 512] int8.

    o[g, plane*16 + nhl, sg*128 + wc*16 + nwl] = parity of dct coeff
    (bh = 1+i', bw = 1+bws, plane = i'*2+bws) of block (c, nh, nw) where
    strip s = 4*g + sg = c*8 + hg, nh = hg*16 + nhl, nw = wc*16 + nwl.
    """
    import concourse.bass as bass
    import concourse.tile as tile
    from concourse import mybir

    f16 = mybir.dt.float16
    f32 = mybir.dt.float32
    i16 = mybir.dt.int16
    nc = bass.Bass()
    # xh/xl rows are HOST-PRESHUFFLED to (k, m) order: element [c, h, k*128+m]
    # holds original pixel [c, h, m*8+k], so each k-slice is contiguous.
    xh = nc.dram_tensor("xh", [C, H, W], f16, kind="ExternalInput")
    xl = nc.dram_tensor("xl", [C, H, W], f16, kind="ExternalInput")
    wc = nc.dram_tensor("wc", [128, 8 * 2 * 128], f16, kind="ExternalInput")
    o = nc.dram_tensor("o", [NSTRIP, 96, 128], i16, kind="ExternalOutput")

    # (channel, first strip-in-channel, n strips): the first channel-half is
    # split into 2-strip groups so compute starts after only 0.5 MB of DMA;
    # the last one too, so the final parity/store tail is short.
    groups = [(0, 0, 2), (0, 2, 2)]
    groups += [(c, hb, 4) for c in range(C) for hb in (0, 4)][1:-1]
    groups += [(2, 4, 2), (2, 6, 2)]

    with tile.TileContext(nc) as tc:
        with (
            tc.tile_pool(name="consts", bufs=1) as consts,
            tc.tile_pool(name="xs", bufs=2 * C) as xpool,
            tc.tile_pool(name="rt", bufs=2) as rpool,
            tc.tile_pool(name="bt", bufs=2) as bpool,
            tc.tile_pool(name="fp", bufs=4, space="PSUM") as fppool,
        ):
            wct = consts.tile([128, 8 * 2 * 128], f16)
            nc.sync.dma_start(out=wct[:], in_=wc[:, :])

            # per-group DMAs in group order, xh before xl, so each group's
            # Xh matmuls can start before its Xl piece has even landed.
            xch, xcl = [], []
            for c in range(C):
                xthc = xpool.tile([128, 8192], f16, tag="xs", name=f"xh{c}")
                xch.append(xthc)
                xtlc = xpool.tile([128, 8192], f16, tag="xs", name=f"xl{c}")
                xcl.append(xtlc)
            for c, hb, n in groups:
                rows = slice(hb * 128, (hb + n) * 128)
                for src, tiles in ((xh, xch), (xl, xcl)):
                    nc.sync.dma_start(
                        out=tiles[c][:].rearrange(
                            "p (t w) -> p t w", t=8)[:, hb:hb + n, :],
                        in_=src[c, rows, :].rearrange("(t p) w -> p t w", p=128))

            def wslice(k, hl):
                return wct[:, (k * 2 + hl) * 128:(k * 2 + hl + 1) * 128]

            s0 = 0
            for c, hb, n in groups:
                fp = fppool.tile([128, 128 * n], f32, tag="fp")
                # channel tile viewed as [128, t=8, k=8, m=128]; the n
                # strips t=hb..hb+n-1 of this group at fixed k give 128*n
                # contiguous-run columns (sg, m).
                xrh = xch[c][:].rearrange("p (t k m) -> p t k m", t=8, k=8)
                xrl = xcl[c][:].rearrange("p (t k m) -> p t k m", t=8, k=8)
                # accumulate Wh*Xh + Wh*Xl + Wl*Xh over all k, Xh passes
                # first (Xl may still be in flight)
                for k in range(8):
                    nc.tensor.matmul(
                        out=fp[:, :], lhsT=wslice(k, 0),
                        rhs=xrh[:, hb:hb + n, k, :],
                        start=(k == 0), stop=False)
                for k in range(8):
                    nc.tensor.matmul(
                        out=fp[:, :], lhsT=wslice(k, 1),
                        rhs=xrh[:, hb:hb + n, k, :],
                        start=False, stop=False)
                for k in range(8):
                    nc.tensor.matmul(
                        out=fp[:, :], lhsT=wslice(k, 0),
                        rhs=xrl[:, hb:hb + n, k, :],
                        start=False, stop=(k == 7))
                # parity: RNE via 3*2^22 magic (sign-safe), then &1
                # (bitVec ops cannot cast, so both tiles are int16)
                rt = rpool.tile([128, 128 * n], i16, tag="rt")
                nc.vector.tensor_scalar(
                    out=rt[0:96, :], in0=fp[0:96, :],
                    scalar1=MAGIC3, scalar2=MAGIC3,
                    op0=mybir.AluOpType.add, op1=mybir.AluOpType.subtract)
                bt = bpool.tile([128, 128 * n], i16, tag="bt")
                nc.vector.tensor_scalar(
                    out=bt[0:96, :], in0=rt[0:96, :],
                    scalar1=1, scalar2=None,
                    op0=mybir.AluOpType.bitwise_and)
                # o[s, q, m] <- bt[q, sg*128 + m] for s = s0 + sg; the DRAM
                # side is permuted to partition-major order to match SBUF
                nc.sync.dma_start(
                    out=o[s0:s0 + n].rearrange("s q m -> q s m"),
                    in_=bt[0:96, :].rearrange("q (s m) -> q s m", s=n))
                s0 += n
    return nc


def build_general_nc(nstrip=NSTRIP):
    """Per-core program: full 64-plane parity table.

    table [nstrip, 128, 1024] f32 where
    table[s=(c,hg), l*16+nwl, wc*128 + nhl*8 + i] =
        parity of dct coeff (bh=i, bw=l) of block (c, hg*16+nhl, wc*16+nwl).
    """
    import concourse.bass as bass
    import concourse.tile as tile
    from concourse import mybir

    f32 = mybir.dt.float32
    nc = bass.Bass()
    x = nc.dram_tensor("x", [C, H, W], f32, kind="ExternalInput")
    br = nc.dram_tensor("br", [128, 128], f32, kind="ExternalInput")
    bc = nc.dram_tensor("bc", [128, 128], f32, kind="ExternalInput")
    o = nc.dram_tensor("o", [nstrip, 128, 1024], f32, kind="ExternalOutput")

    with tile.TileContext(nc) as tc:
        with (
            tc.tile_pool(name="consts", bufs=1) as consts,
            tc.tile_pool(name="xs", bufs=2) as xpool,
            tc.tile_pool(name="ysb", bufs=2) as ypool,
            tc.tile_pool(name="pk", bufs=2) as pkpool,
            tc.tile_pool(name="yp", bufs=4, space="PSUM") as yppool,
            tc.tile_pool(name="fp", bufs=4, space="PSUM") as fppool,
        ):
            brt = consts.tile([128, 128], f32)
            nc.sync.dma_start(out=brt[:], in_=br[:, :])
            bct = consts.tile([128, 128], f32)
            nc.sync.dma_start(out=bct[:], in_=bc[:, :])

            for s in range(nstrip):
                c, hg = divmod(s, H // 128)
                xs = xpool.tile([128, 1024], f32, tag="xs")
                nc.sync.dma_start(
                    out=xs[:], in_=x[c, hg * 128:(hg + 1) * 128, :])
                ysb = ypool.tile([128, 1024], f32, tag="ysb")
                for wc in range(8):
                    yp = yppool.tile([128, 128], f32, tag="yp")
                    nc.tensor.matmul(
                        out=yp[:],
                        lhsT=xs[:, wc * 128:(wc + 1) * 128],
                        rhs=brt[:],
                        start=True, stop=True)
                    nc.vector.tensor_copy(
                        out=ysb[:, wc * 128:(wc + 1) * 128], in_=yp[:])
                pk = pkpool.tile([128, 1024], f32, tag="pk")
                hk = pkpool.tile([128, 1024], f32, tag="hk")
                for wc in range(8):
                    fp = fppool.tile([128, 128], f32, tag="fp")
                    nc.tensor.matmul(
                        out=fp[:],
                        lhsT=bct[:],
                        rhs=ysb[:, wc * 128:(wc + 1) * 128],
                        start=True, stop=True)
                    nc.scalar.activation(
                        out=pk[:, wc * 128:(wc + 1) * 128], in_=fp[:],
                        func=mybir.ActivationFunctionType.Abs)
                _parity_ops(nc, pk, hk)
                nc.sync.dma_start(out=o[s], in_=pk[:])
    return nc


def _run_spmd(nc, in_maps, trace=False):
    from concourse.bass_utils import run_bass_kernel_spmd

    _split_sync_waits(nc)

    res = run_bass_kernel_spmd(
        nc, in_maps, core_ids=list(range(B)), trace=trace)
    _CACHE["last_results"] = res
    return res.results


def _fast_path(stego, trace=False):
    key = "fast_nc"
    if key not in _CACHE:
        _CACHE[key] = build_fast_nc()
    nc = _CACHE[key]
    WC = _CACHE.setdefault("consts_fast", _build_consts_fast())
    Xh = stego.astype(np.float16)
    Xl = (stego - Xh.astype(np.float32)).astype(np.float16)
    # shuffle each row from w = m*8+k order to (k, m) order so the
    # device-side k-slices are contiguous
    Xh = np.ascontiguousarray(
        Xh.reshape(B, C, H, W // 8, 8).swapaxes(-1, -2)).reshape(B, C, H, W)
    Xl = np.ascontiguousarray(
        Xl.reshape(B, C, H, W // 8, 8).swapaxes(-1, -2)).reshape(B, C, H, W)
    in_maps = [
        {"xh": Xh[b], "xl": Xl[b], "wc": WC}
        for b in range(B)
    ]
    results = _run_spmd(nc, in_maps, trace=trace)
    out = np.zeros((B, NUM_BITS), dtype=np.float32)
    for b in range(B):
        O = results[b]["o"]  # [24, 96, 128] int16
        live = O[:, 16:80, :]                       # planes 1..4 -> p 0..3
        a = live.reshape(NSTRIP, 4, 16, 8, 16)      # s, p, nhl, wc, nwl
        a = a.transpose(0, 2, 3, 4, 1)              # s, nhl, wc, nwl, p
        seg = a.reshape(-1).astype(np.float32)      # (c, nh, nw, p) flat
        out[b, b * SEG:(b + 1) * SEG] = seg
    return out


def _general_path(stego, b_idx, c_idx, nh_idx, nw_idx, bh_idx, bw_idx,
                  trace=False):
    key = "general_nc"
    if key not in _CACHE:
        _CACHE[key] = build_general_nc()
    nc = _CACHE[key]
    BR8, BC8 = _CACHE.setdefault("consts_general", _build_consts_general())
    in_maps = [
        {"x": np.ascontiguousarray(stego[b]), "br": BR8, "bc": BC8}
        for b in range(B)
    ]
    results = _run_spmd(nc, in_maps, trace=trace)

    b_idx = np.asarray(b_idx).astype(np.int64)
    c_idx = np.asarray(c_idx).astype(np.int64)
    nh_idx = np.asarray(nh_idx).astype(np.int64)
    nw_idx = np.asarray(nw_idx).astype(np.int64)
    bh_idx = np.asarray(bh_idx).astype(np.int64)
    bw_idx = np.asarray(bw_idx).astype(np.int64)
    num_bits = b_idx.shape[0]

    # table[s=(c,hg), l*16+nwl, wc*128 + nhl*8 + i]
    s = c_idx * 8 + nh_idx // 16
    part = bw_idx * 16 + nw_idx % 16
    free = (nw_idx // 16) * 128 + (nh_idx % 16) * 8 + bh_idx
    flat = (s * 128 + part) * 1024 + free

    out = np.zeros((B, num_bits), dtype=np.float32)
    cols = np.arange(num_bits)
    for b in range(B):
        tb = results[b]["o"].reshape(-1)
        mask = b_idx == b
        out[b, cols[mask]] = tb[flat[mask]]
    return out


def kernel(stego, b_idx, c_idx, nh_idx, nw_idx, bh_idx, bw_idx):
    stego = np.ascontiguousarray(np.asarray(stego, dtype=np.float32))
    import os
    trace = os.environ.get("BASS_TRACE", "") not in ("", "0")
    if _is_canonical(b_idx, c_idx, nh_idx, nw_idx, bh_idx, bw_idx):
        return _fast_path(stego, trace=trace)
    return _general_path(
        stego, b_idx, c_idx, nh_idx, nw_idx, bh_idx, bw_idx, trace=trace)


# revision 23
# speedup vs baseline: 1.0704x; 1.0430x over previous
"""Trainium2 Bass kernel for nn_DCTExtractor.

Reference computation:
  - stego [8, 3, 1024, 1024] f32; per 8x8 block 2D DCT-II (D @ X @ D^T).
  - bits[i] = abs(round_half_even(dct[b,c,nh,nw,bh,bw])) % 2 for 1572864
    index tuples.
  - out [8, num_bits]: out[b_idx[i], i] = bits[i]; other rows 0.

Sharding: data-parallel over batch b across the 8 NeuronCores; core b
processes image b and produces output row b.

Fast path (canonical meshgrid indices: every (b,c,nh,nw) block contributes
positions (1,2),(2,1),(2,2),(3,1)): both DCT stages are fused into ONE PE
pass per 4-strip group via accumulating matmuls (contraction over k, the
column-within-block):

    F[q=(plane,nhl), m=(sg,wc,nwl)] = sum_k Wk^T @ X[:, m*8+k]
      Wk[8*nhl'+j, plane*16+nhl] = delta(nhl,nhl') * D[1+i',j] * D[1+bws,k]
      plane = i'*2 + bws; live planes 1..4 map to p = plane-1.

  The stationary operand is the small constant Wk (96 cols, cheap FWL
  loads); the image streams as the moving operand with free dim 512 (full
  1 cyc/col rate).  This eliminates the per-chunk LDWEIGHTS of image data
  that dominated the 2-stage version, plus the intermediate PSUM->SBUF
  copy and transposes.

  Precision: the PE multiplies at e10m11, so fp32 operands would be
  truncated to ~11-bit mantissas (1e-3 coeff error -> thousands of bit
  flips).  Instead the host splits X and W into fp16 hi+lo pairs
  (11+11-bit mantissas; fp16 products are exact in the e10m23 datapath)
  and the kernel accumulates 3 passes: Wh*Xh + Wh*Xl + Wl*Xh (the dropped
  Wl*Xl term is ~2e-6).  Measured coeff error ~5e-6 -> 0 bit flips.  The
  two fp16 images total the same 12 MiB of DMA as the original f32.

  Parity is 2 DVE ops per group: r = (F + 3*2^22) - 3*2^22 (RNE for
  |F| < 2^22, no abs needed) written as int32, then bits = r & 1 as int8
  (two's complement AND gives |n| mod 2 for negatives too).

General fallback (arbitrary indices): device computes the full 64-plane
parity table per image; host gathers bits and applies the b mask.
"""

import sys

if "/opt/trn_rl_repo" not in sys.path:
    sys.path.insert(0, "/opt/trn_rl_repo")

import numpy as np

BS = 8
B, C, H, W = 8, 3, 1024, 1024
NBH, NBW = H // BS, W // BS
POS = np.array([[1, 2], [2, 1], [2, 2], [3, 1]], dtype=np.int32)
NPOS = 4
SEG = C * NBH * NBW * NPOS  # bits per batch element = 196608
NUM_BITS = B * SEG
NSTRIP = C * (H // 128)  # 24 strips of 128 image rows per image
NGROUP = 6               # 4 strips per group
MAGIC = float(np.float32(8388608.0))    # 2^23 (general path, inputs >= 0)
MAGIC3 = float(np.float32(12582912.0))  # 3*2^22: RNE for |x| < 2^22, any sign

_CACHE = {}


def _split_sync_waits(nc):
    """The staged walrus build accepts at most ONE sync wait per
    instruction, but Tile's wait-assignment freely attaches several.
    Hoist all but the last wait of each instruction onto same-engine
    NoOps inserted directly before it (engines execute their stream in
    order, so the semantics are identical)."""
    from concourse import mybir

    if getattr(nc, "_sync_waits_split", False):
        return
    nc._sync_waits_split = True
    counter = 0
    for bb in nc.m.functions[0].blocks:
        out = []
        changed = False
        for inst in bb.instructions:
            si = inst.sync_info
            waits = list(si.on_wait) if si is not None else []
            if len(waits) > 1:
                for w in waits[:-1]:
                    nop = mybir.InstNoOp(
                        name=f"I-splitw-{counter}", ins=[], outs=[])
                    counter += 1
                    nop.engine = inst.engine
                    nop.sync_info = mybir.SyncInfo(on_update=[], on_wait=[w])
                    out.append(nop)
                si.on_wait = waits[-1:]
                changed = True
            out.append(inst)
        if changed:
            bb.instructions = out
    return


def _dct_matrix_f32() -> np.ndarray:
    k = np.arange(BS)[:, None].astype(np.float64)
    m = np.arange(BS)[None, :].astype(np.float64)
    D = np.cos(np.pi * (2.0 * m + 1.0) * k / (2.0 * BS)) * np.sqrt(2.0 / BS)
    D[0, :] = np.sqrt(1.0 / BS)
    return D.astype(np.float32)


def _canonical_indices():
    b, c, nh, nw, p = np.meshgrid(
        np.arange(B), np.arange(C), np.arange(NBH), np.arange(NBW),
        np.arange(NPOS), indexing="ij")
    return {
        "b_idx": b.reshape(-1).astype(np.int32),
        "c_idx": c.reshape(-1).astype(np.int32),
        "nh_idx": nh.reshape(-1).astype(np.int32),
        "nw_idx": nw.reshape(-1).astype(np.int32),
        "bh_idx": POS[p.reshape(-1), 0].astype(np.int32),
        "bw_idx": POS[p.reshape(-1), 1].astype(np.int32),
    }


def _is_canonical(b_idx, c_idx, nh_idx, nw_idx, bh_idx, bw_idx) -> bool:
    if b_idx.shape[0] != NUM_BITS:
        return False
    canon = _CACHE.setdefault("canon", _canonical_indices())
    got = {"b_idx": b_idx, "c_idx": c_idx, "nh_idx": nh_idx,
           "nw_idx": nw_idx, "bh_idx": bh_idx, "bw_idx": bw_idx}
    return all(np.array_equal(np.asarray(got[k]), canon[k]) for k in canon)


def _build_consts_fast():
    """WC [128, 8, 2, 96] fp16: for each k, hi/lo split of
    Wk[8*nhl+j, plane*16+nhl] = D[1+i',j] * D[1+bws,k] with
    plane = i'*2 + bws (i' in 0..2, bws in 0..1).  Live planes are 1..4
    (p = plane-1); planes 0 and 5 are computed but unused."""
    D = _dct_matrix_f32()
    BRC = np.zeros((128, 8, 96), dtype=np.float32)
    for nhl in range(16):
        for ip in range(3):
            for bws in range(2):
                plane = ip * 2 + bws
                # outer product over (j, k)
                BRC[8 * nhl:8 * nhl + 8, :, plane * 16 + nhl] = (
                    D[1 + ip, :][:, None] * D[1 + bws, :][None, :])
    Wh = BRC.astype(np.float16)
    Wl = (BRC - Wh.astype(np.float32)).astype(np.float16)
    # pad free dim 96 -> 128 so FWL (fast weight load) triggers
    WC = np.zeros((128, 8, 2, 128), dtype=np.float16)
    WC[:, :, 0, :96] = Wh
    WC[:, :, 1, :96] = Wl
    return np.ascontiguousarray(WC.reshape(128, 8 * 2 * 128))


def _build_consts_general():
    D = _dct_matrix_f32()
    BR8 = np.zeros((128, 128), dtype=np.float32)
    for nhl in range(16):
        BR8[nhl * 8:(nhl + 1) * 8, nhl * 8:(nhl + 1) * 8] = D.T  # [j, i]
    BC8 = np.zeros((128, 128), dtype=np.float32)
    for l in range(8):
        for nwl in range(16):
            BC8[nwl * 8:(nwl + 1) * 8, l * 16 + nwl] = D[l, :]
    return BR8, BC8


def _parity_ops(nc, pk, hk):
    """pk holds |coeff|. Compute parity = |RNE(pk)| mod 2 into pk using only
    add/sub/mul/abs (general path):
      r  = (pk + 2^23) - 2^23        round-half-even to integer
      h  = r * 0.5
      rh = (h + 2^23) - 2^23         = h if r even, else nearest even int
      parity = 2*|h - rh|            0.0 or 1.0
    Each step is its own instruction so every intermediate is rounded f32."""
    from concourse import mybir

    ts = nc.vector.tensor_scalar
    add, sub, mult = (mybir.AluOpType.add, mybir.AluOpType.subtract,
                      mybir.AluOpType.mult)
    ts(out=pk[:], in0=pk[:], scalar1=MAGIC, scalar2=None, op0=add)
    ts(out=pk[:], in0=pk[:], scalar1=MAGIC, scalar2=None, op0=sub)
    ts(out=hk[:], in0=pk[:], scalar1=0.5, scalar2=None, op0=mult)
    ts(out=pk[:], in0=hk[:], scalar1=MAGIC, scalar2=None, op0=add)
    ts(out=pk[:], in0=pk[:], scalar1=MAGIC, scalar2=None, op0=sub)
    nc.vector.tensor_tensor(
        out=pk[:], in0=hk[:], in1=pk[:], op=sub)
    nc.scalar.activation(
        out=pk[:], in_=pk[:], func=mybir.ActivationFunctionType.Abs,
        scale=2.0)


def build_fast_nc():
    """Per-core program: xh/xl [3,1024,1024] fp16 -> o [6, 96, 512] int8.

    o[g, plane*16 + nhl, sg*128 + wc*16 + nwl] = parity of dct coeff
    (bh = 1+i', bw = 1+bws, plane = i'*2+bws) of block (c, nh, nw) where
    strip s = 4*g + sg = c*8 + hg, nh = hg*16 + nhl, nw = wc*16 + nwl.
    """
    import concourse.bass as bass
    import concourse.tile as tile
    from concourse import mybir

    f16 = mybir.dt.float16
    f32 = mybir.dt.float32
    i16 = mybir.dt.int16
    nc = bass.Bass()
    # xh/xl rows are HOST-PRESHUFFLED to (k, m) order: element [c, h, k*128+m]
    # holds original pixel [c, h, m*8+k], so each k-slice is contiguous.
    xh = nc.dram_tensor("xh", [C, H, W], f16, kind="ExternalInput")
    xl = nc.dram_tensor("xl", [C, H, W], f16, kind="ExternalInput")
    wc = nc.dram_tensor("wc", [128, 8 * 2 * 128], f16, kind="ExternalInput")
    o = nc.dram_tensor("o", [NSTRIP, 96, 128], i16, kind="ExternalOutput")

    # (channel, first strip-in-channel, n strips): the first channel-half is
    # split into 2-strip groups so compute starts after only 0.5 MB of DMA;
    # the last one too, so the final parity/store tail is short.
    groups = [(0, 0, 2), (0, 2, 2)]
    groups += [(c, hb, 4) for c in range(C) for hb in (0, 4)][1:-1]
    groups += [(2, 4, 2), (2, 6, 2)]

    with tile.TileContext(nc) as tc:
        with (
            tc.tile_pool(name="consts", bufs=1) as consts,
            tc.tile_pool(name="xs", bufs=2 * C) as xpool,
            tc.tile_pool(name="rt", bufs=4) as rpool,
            tc.tile_pool(name="bt", bufs=4) as bpool,
            tc.tile_pool(name="fp", bufs=4, space="PSUM") as fppool,
        ):
            wct = consts.tile([128, 8 * 2 * 128], f16)
            nc.sync.dma_start(out=wct[:], in_=wc[:, :])

            # per-group DMAs in group order, xh before xl, so each group's
            # Xh matmuls can start before its Xl piece has even landed.
            xch, xcl = [], []
            for c in range(C):
                xthc = xpool.tile([128, 8192], f16, tag="xs", name=f"xh{c}")
                xch.append(xthc)
                xtlc = xpool.tile([128, 8192], f16, tag="xs", name=f"xl{c}")
                xcl.append(xtlc)
            for c, hb, n in groups:
                rows = slice(hb * 128, (hb + n) * 128)
                for src, tiles in ((xh, xch), (xl, xcl)):
                    nc.sync.dma_start(
                        out=tiles[c][:].rearrange(
                            "p (t w) -> p t w", t=8)[:, hb:hb + n, :],
                        in_=src[c, rows, :].rearrange("(t p) w -> p t w", p=128))

            def wslice(k, hl):
                return wct[:, (k * 2 + hl) * 128:(k * 2 + hl + 1) * 128]

            s0 = 0
            for c, hb, n in groups:
                fp = fppool.tile([128, 128 * n], f32, tag="fp")
                # channel tile viewed as [128, t=8, k=8, m=128]; the n
                # strips t=hb..hb+n-1 of this group at fixed k give 128*n
                # contiguous-run columns (sg, m).
                xrh = xch[c][:].rearrange("p (t k m) -> p t k m", t=8, k=8)
                xrl = xcl[c][:].rearrange("p (t k m) -> p t k m", t=8, k=8)
                # accumulate Wh*Xh + Wh*Xl + Wl*Xh over all k, Xh passes
                # first (Xl may still be in flight)
                for k in range(8):
                    nc.tensor.matmul(
                        out=fp[:, :], lhsT=wslice(k, 0),
                        rhs=xrh[:, hb:hb + n, k, :],
                        start=(k == 0), stop=False)
                for k in range(8):
                    nc.tensor.matmul(
                        out=fp[:, :], lhsT=wslice(k, 1),
                        rhs=xrh[:, hb:hb + n, k, :],
                        start=False, stop=False)
                for k in range(8):
                    nc.tensor.matmul(
                        out=fp[:, :], lhsT=wslice(k, 0),
                        rhs=xrl[:, hb:hb + n, k, :],
                        start=False, stop=(k == 7))
                # parity: RNE via 3*2^22 magic (sign-safe), then &1
                # (bitVec ops cannot cast, so both tiles are int16)
                rt = rpool.tile([128, 128 * n], i16, tag="rt")
                nc.vector.tensor_scalar(
                    out=rt[0:96, :], in0=fp[0:96, :],
                    scalar1=MAGIC3, scalar2=MAGIC3,
                    op0=mybir.AluOpType.add, op1=mybir.AluOpType.subtract)
                bt = bpool.tile([128, 128 * n], i16, tag="bt")
                nc.vector.tensor_scalar(
                    out=bt[0:96, :], in0=rt[0:96, :],
                    scalar1=1, scalar2=None,
                    op0=mybir.AluOpType.bitwise_and)
                # o[s, q, m] <- bt[q, sg*128 + m] for s = s0 + sg; the DRAM
                # side is permuted to partition-major order to match SBUF
                nc.sync.dma_start(
                    out=o[s0:s0 + n].rearrange("s q m -> q s m"),
                    in_=bt[0:96, :].rearrange("q (s m) -> q s m", s=n))
                s0 += n
    return nc


def build_general_nc(nstrip=NSTRIP):
    """Per-core program: full 64-plane parity table.

    table [nstrip, 128, 1024] f32 where
    table[s=(c,hg), l*16+nwl, wc*128 + nhl*8 + i] =
        parity of dct coeff (bh=i, bw=l) of block (c, hg*16+nhl, wc*16+nwl).
    """
    import concourse.bass as bass
    import concourse.tile as tile
    from concourse import mybir

    f32 = mybir.dt.float32
    nc = bass.Bass()
    x = nc.dram_tensor("x", [C, H, W], f32, kind="ExternalInput")
    br = nc.dram_tensor("br", [128, 128], f32, kind="ExternalInput")
    bc = nc.dram_tensor("bc", [128, 128], f32, kind="ExternalInput")
    o = nc.dram_tensor("o", [nstrip, 128, 1024], f32, kind="ExternalOutput")

    with tile.TileContext(nc) as tc:
        with (
            tc.tile_pool(name="consts", bufs=1) as consts,
            tc.tile_pool(name="xs", bufs=2) as xpool,
            tc.tile_pool(name="ysb", bufs=2) as ypool,
            tc.tile_pool(name="pk", bufs=2) as pkpool,
            tc.tile_pool(name="yp", bufs=4, space="PSUM") as yppool,
            tc.tile_pool(name="fp", bufs=4, space="PSUM") as fppool,
        ):
            brt = consts.tile([128, 128], f32)
            nc.sync.dma_start(out=brt[:], in_=br[:, :])
            bct = consts.tile([128, 128], f32)
            nc.sync.dma_start(out=bct[:], in_=bc[:, :])

            for s in range(nstrip):
                c, hg = divmod(s, H // 128)
                xs = xpool.tile([128, 1024], f32, tag="xs")
                nc.sync.dma_start(
                    out=xs[:], in_=x[c, hg * 128:(hg + 1) * 128, :])
                ysb = ypool.tile([128, 1024], f32, tag="ysb")
                for wc in range(8):
                    yp = yppool.tile([128, 128], f32, tag="yp")
                    nc.tensor.matmul(
                        out=yp[:],
                        lhsT=xs[:, wc * 128:(wc + 1) * 128],
                        rhs=brt[:],
                        start=True, stop=True)
                    nc.vector.tensor_copy(
                        out=ysb[:, wc * 128:(wc + 1) * 128], in_=yp[:])
                pk = pkpool.tile([128, 1024], f32, tag="pk")
                hk = pkpool.tile([128, 1024], f32, tag="hk")
                for wc in range(8):
                    fp = fppool.tile([128, 128], f32, tag="fp")
                    nc.tensor.matmul(
                        out=fp[:],
                        lhsT=bct[:],
                        rhs=ysb[:, wc * 128:(wc + 1) * 128],
                        start=True, stop=True)
                    nc.scalar.activation(
                        out=pk[:, wc * 128:(wc + 1) * 128], in_=fp[:],
                        func=mybir.ActivationFunctionType.Abs)
                _parity_ops(nc, pk, hk)
                nc.sync.dma_start(out=o[s], in_=pk[:])
    return nc


def _run_spmd(nc, in_maps, trace=False):
    from concourse.bass_utils import run_bass_kernel_spmd

    _split_sync_waits(nc)

    res = run_bass_kernel_spmd(
        nc, in_maps, core_ids=list(range(B)), trace=trace)
    _CACHE["last_results"] = res
    return res.results


def _fast_path(stego, trace=False):
    key = "fast_nc"
    if key not in _CACHE:
        _CACHE[key] = build_fast_nc()
    nc = _CACHE[key]
    WC = _CACHE.setdefault("consts_fast", _build_consts_fast())
    Xh = stego.astype(np.float16)
    Xl = (stego - Xh.astype(np.float32)).astype(np.float16)
    # shuffle each row from w = m*8+k order to (k, m) order so the
    # device-side k-slices are contiguous
    Xh = np.ascontiguousarray(
        Xh.reshape(B, C, H, W // 8, 8).swapaxes(-1, -2)).reshape(B, C, H, W)
    Xl = np.ascontiguousarray(
        Xl.reshape(B, C, H, W // 8, 8).swapaxes(-1, -2)).reshape(B, C, H, W)
    in_maps = [
        {"xh": Xh[b], "xl": Xl[b], "wc": WC}
        for b in range(B)
    ]
    results = _run_spmd(nc, in_maps, trace=trace)
    out = np.zeros((B, NUM_BITS), dtype=np.float32)
    for b in range(B):
        O = results[b]["o"]  # [24, 96, 128] int16
        live = O[:, 16:80, :]                       # planes 1..4 -> p 0..3
        a = live.reshape(NSTRIP, 4, 16, 8, 16)      # s, p, nhl, wc, nwl
        a = a.transpose(0, 2, 3, 4, 1)              # s, nhl, wc, nwl, p
        seg = a.reshape(-1).astype(np.float32)      # (c, nh, nw, p) flat
        out[b, b * SEG:(b + 1) * SEG] = seg
    return out


def _general_path(stego, b_idx, c_idx, nh_idx, nw_idx, bh_idx, bw_idx,
                  trace=False):
    key = "general_nc"
    if key not in _CACHE:
        _CACHE[key] = build_general_nc()
    nc = _CACHE[key]
    BR8, BC8 = _CACHE.setdefault("consts_general", _build_consts_general())
    in_maps = [
        {"x": np.ascontiguousarray(stego[b]), "br": BR8, "bc": BC8}
        for b in range(B)
    ]
    results = _run_spmd(nc, in_maps, trace=trace)

    b_idx = np.asarray(b_idx).astype(np.int64)
    c_idx = np.asarray(c_idx).astype(np.int64)
    nh_idx = np.asarray(nh_idx).astype(np.int64)
    nw_idx = np.asarray(nw_idx).astype(np.int64)
    bh_idx = np.asarray(bh_idx).astype(np.int64)
    bw_idx = np.asarray(bw_idx).astype(np.int64)
    num_bits = b_idx.shape[0]

    # table[s=(c,hg), l*16+nwl, wc*128 + nhl*8 + i]
    s = c_idx * 8 + nh_idx // 16
    part = bw_idx * 16 + nw_idx % 16
    free = (nw_idx // 16) * 128 + (nh_idx % 16) * 8 + bh_idx
    flat = (s * 128 + part) * 1024 + free

    out = np.zeros((B, num_bits), dtype=np.float32)
    cols = np.arange(num_bits)
    for b in range(B):
        tb = results[b]["o"].reshape(-1)
        mask = b_idx == b
        out[b, cols[mask]] = tb[flat[mask]]
    return out


def kernel(stego, b_idx, c_idx, nh_idx, nw_idx, bh_idx, bw_idx):
    stego = np.ascontiguousarray(np.asarray(stego, dtype=np.float32))
    import os
    trace = os.environ.get("BASS_TRACE", "") not in ("", "0")
    if _is_canonical(b_idx, c_idx, nh_idx, nw_idx, bh_idx, bw_idx):
        return _fast_path(stego, trace=trace)
    return _general_path(
        stego, b_idx, c_idx, nh_idx, nw_idx, bh_idx, bw_idx, trace=trace)
